# revision 34
# baseline (speedup 1.0000x reference)
"""Trainium2 Bass kernel for nn_AdaptBlockV2 (deformable-conv-v2 block).

Data-parallel over the batch axis: 8 samples -> 8 NeuronCores, one sample
per core. Inside each core:
  A) load x; build zero-padded CHW copy (bf16) for the convs; transpose x to
     HWC (bf16, three band tiles) and write a ROW-MAJOR "quad" gather table
     straight to DRAM with 4 strided DMAs per band (row r = channels of flat
     pixels [r, r+1, r+W, r+W+1] at slots 0..3) -- one indirect-DMA
     descriptor then fetches all 4 bilinear corners of one (pixel, tap).
     Table edge rows are pre-zeroed so zero-weight fetches read 0, not NaN.
  B) 15-channel 3x3 conv (offset transform T, translation tr, modulation
     mask) as 9 PSUM-accumulated matmuls; transpose conv output to
     pixel-major; bulk DVE math for sampling positions py/px, floor via
     floored-mod, corner weights (bilinear x mask x validity), and the flat
     gather index (= table row, no remap needed with the row-major table).
  C) per-slice pipeline: indirect DMA gather -> DVE weighted 4-corner
     combine -> PE transpose of samp to (tap,channel)-major -> matmul with
     dw -> BN (running stats) + residual (DVE add) + ReLU (Scalar engine)
     -> DMA out.

The SWDGE gather descriptor generation on the Pool engine (~8ns/descriptor
x 62208 descriptors ~= 490us) is the hard floor; the prologue is kept to
~25us by minimizing serial Sync-engine DMA issues (the quad table is 10
DMAs instead of ~72; the idx 16-wrap fold is 3 DMAs per group instead of 9
-- queue 0 only needs the indices replicated to partitions 0..31).

kernel(**inputs) takes FULL unsharded inputs, returns the FULL output.
"""
import numpy as np
import ml_dtypes

N, C, H, W = 8, 48, 96, 72
HW = H * W                       # 6912
LEAD = W + 2                     # 74: lead pad rows in the quad table
RQ = 7040                        # quad-table rows (>= HW + W + 2)
QW = 256                         # quad-table row width (512B, dma_gather)
NB = HW // 128                   # 54 pixel blocks
QTOT = NB * 9                    # 486 (block, tap) chunks
PADW = W + 2                     # 74 padded conv row stride
PADLEN = (H + 2) * PADW         # 7252
BN_EPS = 1e-5
CONV_ROWS = 7                    # conv N-tile = 7 image rows = 504 pixels
SLICE_BLOCKS = 4                 # gather/combine slice = 4 pixel blocks
MAXD = 10                        # sample-displacement bound for band deps
BANDS = [(0, 12), (12, 32), (32, 54)]   # x_hwc chunk ranges per table band
DELTA = [0, 1, W, W + 1]                # quad slot pixel offsets

_REG = np.array([[-1, -1, -1, 0, 0, 0, 1, 1, 1],
                 [-1, 0, 1, -1, 0, 1, -1, 0, 1]], dtype=np.float32)

_built = {}


def _slices():
    # small slices at BOTH ends: early ones make the first combines (and
    # the gq-buffer WAR they release) come sooner so the multi-queue
    # pipeline ramps up fast; late ones stagger the final combine tails
    plan = [2] * 4 + [4] * 10 + [2] * 3
    out = []
    b = 0
    for nb in plan:
        out.append((b, nb))
        b += nb
    assert b == NB
    return out


def build_nc():
    import concourse.bass as bass
    import concourse.bacc as bacc
    import concourse.tile as tile
    from concourse import mybir
    from concourse.bass import AP
    from concourse.masks import make_identity
    from concourse.tile import add_dep_helper
    from contextlib import ExitStack

    dt = mybir.dt
    op = mybir.AluOpType
    act = mybir.ActivationFunctionType

    nc = bacc.Bacc("TRN2", target_bir_lowering=False, debug=False,
                   num_devices=N, dynamic_dma_scratch_size=16384,
                   num_swdge_queues=4)
    x_ext = nc.declare_dram_parameter("x", [C, HW], dt.bfloat16, isOutput=False)
    blob16_ext = nc.declare_dram_parameter("blob16", [128, 282], dt.bfloat16,
                                           isOutput=False)
    blob32_ext = nc.declare_dram_parameter("blob32", [128, 131], dt.float32,
                                           isOutput=False)
    out_ext = nc.declare_dram_parameter("out", [C, HW], dt.float32, isOutput=True)

    x_quad = nc.dram_tensor("x_quad", [RQ, QW], dt.bfloat16)
    idx_dram = nc.dram_tensor("idx_dram", [128 * QTOT], dt.int16)

    with tile.TileContext(nc) as tc, ExitStack() as ctx:
        cp = ctx.enter_context(tc.tile_pool(name="const", bufs=1))
        tp = ctx.enter_context(tc.tile_pool(name="tmp", bufs=1))
        wp = ctx.enter_context(tc.tile_pool(name="work", bufs=2))
        pp_a = ctx.enter_context(tc.tile_pool(name="ps_a", bufs=2, space="PSUM"))
        pp_st = ctx.enter_context(tc.tile_pool(name="ps_st", bufs=2, space="PSUM"))
        pp_out = ctx.enter_context(tc.tile_pool(name="ps_out", bufs=1, space="PSUM"))

        # ---------------- constants / weights to SBUF ----------------
        x_sb = cp.tile([C, HW], dt.bfloat16, tag="x_sb")
        nc.sync.dma_start(x_sb[:, :HW // 2], x_ext[:, :HW // 2])
        nc.sync.dma_start(x_sb[:, HW // 2:], x_ext[:, HW // 2:])
        blob16 = cp.tile([128, 282], dt.bfloat16, tag="blob16")
        nc.sync.dma_start(blob16[:], blob16_ext[:])
        blob32 = cp.tile([128, 131], dt.float32, tag="blob32")
        nc.sync.dma_start(blob32[:], blob32_ext[:])

        b16p = blob16[:].ap[0][0]
        b16o = blob16[:].offset
        b32p = blob32[:].ap[0][0]
        b32o = blob32[:].offset

        def wconvA_ap(r):        # [96, 15] stationary: taps (r,0) + (r,1)
            return AP(blob16.tensor, b16o + r * 15, [[b16p, 2 * C], [1, 15]])

        def wconvB_ap(r):        # [48, 15] stationary: tap (r,2)
            return AP(blob16.tensor, b16o + 45 + r * 15, [[b16p, C], [1, 15]])

        def dwt_ap(ch, nparts):  # [nparts, 48] stationary for dw chunk ch
            return AP(blob16.tensor, b16o + 90 + ch * C, [[b16p, nparts], [1, C]])

        def b32col(col, nparts):
            return AP(blob32.tensor, b32o + col, [[b32p, nparts], [1, 1]])

        bconv_ap = b32col(126, 15)

        id128 = cp.tile([128, 128], dt.bfloat16, tag="id128")
        make_identity(nc, id128[:])
        id48 = id128[0:C, 0:C]
        id16 = id128[0:15, 0:15]

        # Dummy 128-descriptor gather issued first: forces the Pool engine's
        # gather-ucode LOAD_LIB (which barriers on all previously emitted
        # work) to happen immediately against an empty pipeline instead of
        # right before the first real gather.
        dz_idx = tp.tile([128, 8], dt.int16, tag="dz_idx")
        nc.gpsimd.memset(dz_idx[:], 0)
        dz_out = tp.tile([128, 256], dt.bfloat16, tag="dz_out")
        nc.gpsimd.dma_gather(
            out_ap=AP(dz_out.tensor, dz_out[:].offset,
                      [dz_out[:].ap[0], [QW, 1], [1, QW]]),
            in_ap=x_quad[:], idxs_ap=dz_idx[:],
            num_idxs=128, num_idxs_reg=128,
            elem_size=QW, single_packet=False)

        # ---------------- zero the quad-table edge rows ----------------
        # Rows [0, LEAD) and [RQ-127, RQ) can be fetched (clamped idx /
        # out-of-image corners) with zero weight; they must hold 0, not junk.
        ztile = tp.tile([128, 254], dt.bfloat16, tag="ztile")
        nc.vector.memset(ztile[:], 0.0)
        zlow = nc.sync.dma_start(
            out=AP(x_quad, 0, [[148, 128], [1, 148]]),
            in_=ztile[:, :148])
        zhigh = nc.sync.dma_start(
            out=AP(x_quad, (RQ - 127) * QW, [[254, 128], [1, 254]]),
            in_=ztile[:, :254])

        # ---------------- padded CHW copy (bf16) for convs ----------------
        # x_pad2 partitions 0..47 hold padded x; partitions 48..95 hold the
        # same shifted one column left (tap c=1), so one matmul covers two
        # taps with a [96, 15] stationary. Only pad cells are memset.
        x_pad2 = cp.tile([2 * C, PADLEN], dt.bfloat16, tag="x_pad2")
        xp_p = x_pad2[:].ap[0][0]
        xp_o = x_pad2[:].offset
        nc.vector.memset(x_pad2[0:C, 0:PADW + 1], 0.0)
        nc.vector.memset(
            AP(x_pad2.tensor, xp_o + 2 * PADW - 1,
               [[xp_p, C], [PADW, H - 1], [1, 2]]), 0.0)
        nc.vector.memset(x_pad2[0:C, (H + 1) * PADW - 1:PADLEN], 0.0)
        nc.vector.tensor_copy(
            AP(x_pad2.tensor, xp_o + PADW + 1, [[xp_p, C], [PADW, H // 2], [1, W]]),
            x_sb[:, :HW // 2])
        nc.vector.tensor_copy(
            AP(x_pad2.tensor, xp_o + (H // 2 + 1) * PADW + 1,
               [[xp_p, C], [PADW, H // 2], [1, W]]),
            x_sb[:, HW // 2:])
        nc.sync.dma_start(
            out=AP(x_pad2.tensor, xp_o + C * xp_p,
                   [[xp_p, C], [1, PADLEN - 1]]),
            in_=AP(x_pad2.tensor, xp_o + 1, [[xp_p, C], [1, PADLEN - 1]]))

        # ---------------- x -> HWC (bf16) via PE transposes ----------------
        # One tile per table band so band writes only dep on their own chunks.
        hwc_tiles = [cp.tile([128, (k1 - k0) * C], dt.bfloat16,
                             name=f"x_hwc{i}", tag=f"x_hwc{i}")
                     for i, (k0, k1) in enumerate(BANDS)]

        def hwc_of(chunk):
            for (k0, k1), t in zip(BANDS, hwc_tiles):
                if k0 <= chunk < k1:
                    return t, k0
            raise AssertionError

        def emit_trans(g):                    # 4 blocks per PSUM tile
            nblk = min(4, NB - g * 4)
            ps = pp_a.tile([128, 4 * C], dt.bfloat16, name="psA", tag="psA")
            for j in range(nblk):
                b = g * 4 + j
                nc.tensor.transpose(ps[:, j * C:(j + 1) * C],
                                    x_sb[:, b * 128:(b + 1) * 128], id48[:])
            t, k0 = hwc_of(g * 4)
            nc.scalar.activation(
                t[:, (g * 4 - k0) * C:(g * 4 - k0 + nblk) * C],
                ps[:, :nblk * C], act.Copy)

        # ---------------- quad table: 4 slot DMAs per band -----------------
        band_ins = {}

        def emit_band(bi):
            k0, k1 = BANDS[bi]
            nk = k1 - k0
            t, tk0 = hwc_of(k0)
            ws = []
            for j, dj in enumerate(DELTA):
                base = (LEAD - dj) * QW + 64 * j
                w = nc.sync.dma_start(
                    out=AP(x_quad, base + k0 * 128 * QW,
                           [[QW, 128], [128 * QW, nk], [1, C]]),
                    in_=AP(t.tensor, t[:].offset + (k0 - tk0) * C,
                           [t[:].ap[0], [C, nk], [1, C]]))
                if bi == 0:
                    add_dep_helper(w.ins, zlow.ins, reason="slot after zero")
                if bi == 2:
                    add_dep_helper(w.ins, zhigh.ins, reason="slot after zero")
                ws.append(w)
            band_ins[bi] = ws

        def _slice_table_deps(b0, nb):
            y_lo = (b0 * 128) // W
            y_hi = ((b0 + nb) * 128 - 1) // W
            r_lo = max(0, LEAD + (y_lo - MAXD) * W)
            r_hi = min(RQ - 1, LEAD + (y_hi + MAXD + 1) * W + W - 1)
            deps = []
            if r_lo < LEAD:
                deps.append(zlow)
            if r_hi >= RQ - 127:
                deps.append(zhigh)
            for bi, (k0, k1) in enumerate(BANDS):
                if r_lo <= k1 * 128 + W + 1 and r_hi >= k0 * 128 + 1:
                    deps.extend(band_ins[bi])
            return deps

        # bn scale' = gamma * rsqrt(rvar+eps); shift' = beta - rmean*scale'
        def emit_bn():
            veps = tp.tile([C, 1], dt.float32, tag="veps")
            nc.vector.tensor_scalar(veps[:], b32col(130, C), BN_EPS, None, op.add)
            vsq = tp.tile([C, 1], dt.float32, tag="vsq")
            nc.scalar.activation(vsq[:], veps[:], act.Sqrt)
            vri = tp.tile([C, 1], dt.float32, tag="vri")
            nc.vector.reciprocal(vri[:], vsq[:])
            scale_t = cp.tile([C, 1], dt.float32, tag="scale")
            nc.vector.tensor_tensor(scale_t[:], b32col(127, C), vri[:], op.mult)
            vms = tp.tile([C, 1], dt.float32, tag="vms")
            nc.vector.tensor_tensor(vms[:], b32col(129, C), scale_t[:], op.mult)
            shift_t = cp.tile([C, 1], dt.float32, tag="shift")
            nc.vector.tensor_tensor(shift_t[:], b32col(128, C), vms[:], op.subtract)
            return scale_t, shift_t

        # ---------------- convs: 15ch 3x3 via 6 accumulated matmuls --------
        conv_sb = cp.tile([15, HW], dt.bfloat16, tag="conv_sb")
        trows = [(t * CONV_ROWS, min(CONV_ROWS, H - t * CONV_ROWS))
                 for t in range((H + CONV_ROWS - 1) // CONV_ROWS)]

        def conv_tile(r0, nr):
            psc = pp_a.tile([15, CONV_ROWS * W], dt.float32, name="psA2",
                            tag="psA2", bufs=3)
            npx = nr * W
            for r in range(3):
                rhsA = AP(x_pad2.tensor, xp_o + (r0 + r) * PADW,
                          [[xp_p, 2 * C], [PADW, nr], [1, W]])
                nc.tensor.matmul(psc[:, :npx], wconvA_ap(r), rhsA,
                                 start=(r == 0), stop=False)
                rhsB = AP(x_pad2.tensor, xp_o + (r0 + r) * PADW + 2,
                          [[xp_p, C], [PADW, nr], [1, W]])
                nc.tensor.matmul(psc[:, :npx], wconvB_ap(r), rhsB,
                                 start=False, stop=(r == 2))
            nc.scalar.activation(conv_sb[:, r0 * W:r0 * W + npx], psc[:, :npx],
                                 act.Identity, bias=bconv_ap)

        # conv output -> pixel-major (tcols), per 8-block group
        tcols = cp.tile([128, NB * 15], dt.float32, tag="tcols")

        def tcols_group(g):
            nblk = min(8, NB - g * 8)
            ps = pp_a.tile([128, 8 * 16], dt.bfloat16, name="psA", tag="psA")
            for j in range(nblk):
                b = g * 8 + j
                nc.tensor.transpose(ps[:, j * 16:j * 16 + 15],
                                    conv_sb[:, b * 128:(b + 1) * 128],
                                    id16[:])
            nc.scalar.activation(tcols[:, g * 8 * 15:(g * 8 + nblk) * 15],
                                 AP(ps.tensor, ps[:].offset,
                                    [ps[:].ap[0], [16, nblk], [1, 15]]),
                                 act.Copy)

        # ---------------- bulk offset / weight / index math ----------------
        def plane(tag):
            return tp.tile([128, QTOT], dt.float32, name=tag, tag=tag)

        py = plane("py"); px = plane("px")
        t1 = plane("t1"); t2 = plane("t2"); t3 = plane("t3")
        fy = plane("fy"); fx = plane("fx")
        y0 = plane("y0"); x0 = plane("x0")
        wy0 = py; wy1 = px                       # reuse dead slots (disjoint cols)
        wx0 = plane("wx0"); wx1 = plane("wx1")
        idxf = fy                                # reuse dead slot
        cvt_i = tp.tile([128, QTOT], dt.int32, name="cvt_i", tag="cvt_i")
        w4 = cp.tile([128, QTOT * 4], dt.bfloat16, tag="w4")
        idx_t = cp.tile([128, QTOT], dt.int16, tag="idx")
        fold = cp.tile([16, 8 * QTOT], dt.int16, tag="fold")
        # one idxw tile per offset-math group, so a slice gather depends only
        # on its own group's fold (Tile tracks whole-tile versions)
        idxw_tiles = {}
        group_repls = []
        group_ilvs = []
        groups = [(0, 4), (4, 4), (8, 8), (16, 16), (32, 22)]

        def emit_group(g0, gn):
            nq = gn * 9
            c0 = g0 * 9
            sl = slice(c0, c0 + nq)

            def tcol_b(ch):
                return AP(tcols.tensor, tcols[:].offset + ch + g0 * 15,
                          [tcols[:].ap[0], [15, gn], [0, 9]])

            def reg_b(roff):
                return AP(blob32.tensor, b32o + roff,
                          [[b32p, 128], [0, gn], [1, 9]])

            def coord_b(coff):
                return AP(blob32.tensor, b32o + coff + g0,
                          [[b32p, 128], [1, gn], [0, 9]])

            tt = nc.vector.tensor_tensor
            ts = nc.vector.tensor_scalar
            tt(t1[:, sl], reg_b(0), tcol_b(0), op.mult)
            tt(t2[:, sl], reg_b(9), tcol_b(1), op.mult)
            tt(t3[:, sl], t1[:, sl], t2[:, sl], op.add)
            tt(t1[:, sl], t3[:, sl], tcol_b(4), op.add)
            tt(py[:, sl], t1[:, sl], coord_b(18), op.add)
            tt(t1[:, sl], reg_b(0), tcol_b(2), op.mult)
            tt(t2[:, sl], reg_b(9), tcol_b(3), op.mult)
            tt(t3[:, sl], t1[:, sl], t2[:, sl], op.add)
            tt(t1[:, sl], t3[:, sl], tcol_b(5), op.add)
            tt(px[:, sl], t1[:, sl], coord_b(72), op.add)

            # floor(v) = int(v) - (v < int(v)); rounding-mode agnostic
            for (v, fl, fr) in ((py, y0, fy), (px, x0, fx)):
                nc.vector.tensor_copy(cvt_i[:, sl], v[:, sl])
                nc.vector.tensor_copy(t1[:, sl], cvt_i[:, sl])
                tt(t2[:, sl], v[:, sl], t1[:, sl], op.is_lt)
                tt(fl[:, sl], t1[:, sl], t2[:, sl], op.subtract)
                tt(fr[:, sl], v[:, sl], fl[:, sl], op.subtract)

            mk_b = AP(tcols.tensor, tcols[:].offset + 6 + g0 * 15,
                      [tcols[:].ap[0], [15, gn], [1, 9]])
            ts(t1[:, sl], y0[:, sl], 0.0, None, op.is_ge)
            ts(t2[:, sl], y0[:, sl], float(H - 1), None, op.is_le)
            tt(t3[:, sl], t1[:, sl], t2[:, sl], op.mult)
            ts(t1[:, sl], fy[:, sl], -1.0, 1.0, op.mult, op.add)
            tt(wy0[:, sl], t1[:, sl], t3[:, sl], op.mult)
            ts(t1[:, sl], y0[:, sl], -1.0, None, op.is_ge)
            ts(t2[:, sl], y0[:, sl], float(H - 2), None, op.is_le)
            tt(t3[:, sl], t1[:, sl], t2[:, sl], op.mult)
            tt(wy1[:, sl], fy[:, sl], t3[:, sl], op.mult)
            ts(t1[:, sl], x0[:, sl], 0.0, None, op.is_ge)
            ts(t2[:, sl], x0[:, sl], float(W - 1), None, op.is_le)
            tt(t3[:, sl], t1[:, sl], t2[:, sl], op.mult)
            ts(t1[:, sl], fx[:, sl], -1.0, 1.0, op.mult, op.add)
            tt(wx0[:, sl], t1[:, sl], t3[:, sl], op.mult)
            ts(t1[:, sl], x0[:, sl], -1.0, None, op.is_ge)
            ts(t2[:, sl], x0[:, sl], float(W - 2), None, op.is_le)
            tt(t3[:, sl], t1[:, sl], t2[:, sl], op.mult)
            tt(wx1[:, sl], fx[:, sl], t3[:, sl], op.mult)
            tt(wy0[:, sl], wy0[:, sl], mk_b, op.mult)
            tt(wy1[:, sl], wy1[:, sl], mk_b, op.mult)

            def w4_slot(j):
                return AP(w4.tensor, w4[:].offset + j + c0 * 4,
                          [w4[:].ap[0], [4, nq]])
            tt(w4_slot(0), wy0[:, sl], wx0[:, sl], op.mult)
            tt(w4_slot(1), wy0[:, sl], wx1[:, sl], op.mult)
            tt(w4_slot(2), wy1[:, sl], wx0[:, sl], op.mult)
            tt(w4_slot(3), wy1[:, sl], wx1[:, sl], op.mult)

            # gather index = table row r = clip(LEAD + y0*W + x0, 0, RQ-1)
            nc.vector.scalar_tensor_tensor(idxf[:, sl], y0[:, sl], float(W),
                                           x0[:, sl], op.mult, op.add)
            ts(idxf[:, sl], idxf[:, sl], float(LEAD), 0.0, op.add, op.max)
            ts(idxf[:, sl], idxf[:, sl], float(RQ - 1), None, op.min)
            nc.vector.tensor_copy(idx_t[:, sl], idxf[:, sl])

            # 16-wrap fold DRAM round-trip (Sync only; DVE interleave comes
            # later, after every group's math, so it never stalls the stream)
            wr = nc.sync.dma_start(
                out=AP(idx_dram, c0, [[QTOT, 128], [1, nq]]),
                in_=idx_t[:, sl])
            rdf = nc.sync.dma_start(
                out=AP(fold.tensor, fold[:].offset + c0,
                       [fold[:].ap[0], [QTOT, 8], [1, nq]]),
                in_=AP(idx_dram, c0, [[QTOT, 16], [16 * QTOT, 8], [1, nq]]))
            add_dep_helper(rdf.ins, wr.ins, sync=False, reason="idx fold rt")

        def emit_group_fold(g0, gn):
            nq = gn * 9
            c0 = g0 * 9
            idxw = cp.tile([128, nq * 8], dt.int16, name=f"idxw{g0}",
                           tag=f"idxw{g0}")
            idxw_tiles[g0] = (g0, gn, idxw)
            i16 = idxw[0:16, :]
            ic = nc.gpsimd.tensor_copy(
                AP(i16.tensor, i16.offset,
                   [i16.ap[0], [1, 8], [8, nq]]),
                AP(fold.tensor, fold[:].offset + c0,
                   [fold[:].ap[0], [QTOT, 8], [1, nq]]))
            group_ilvs.append(ic)
            # queue q reads idx partitions [32q, 32q+32): replicate the
            # 16-row wrap to all of 0..127 with 7 parallel DMAs
            for k in range(1, 8):
                r = nc.sync.dma_start(out=idxw[16 * k:16 * (k + 1), :],
                                      in_=idxw[0:16, :])
            group_repls.append(r)

        # ---------------- emission schedule ----------------
        # All offset-math groups complete BEFORE the first gather launches:
        # DVE ops that execute while a Pool gather is in flight can stall
        # until the gather retires (observed on HW), so the gather window is
        # kept to combine/epilogue ops that are proven immune. PE runs the
        # convs first (they feed tcols -> group math), then the x_hwc
        # transposes that feed the table-band writes.
        for g in range(3):
            emit_trans(g)
        emit_band(0)
        conv_tile(*trows[0]); conv_tile(*trows[1]); conv_tile(*trows[2])
        tcols_group(0)
        emit_group(0, 4)
        emit_group_fold(0, 4)
        emit_group(4, 4)
        emit_group_fold(4, 4)
        scale_t, shift_t = emit_bn()
        conv_tile(*trows[3]); conv_tile(*trows[4])
        tcols_group(1)
        emit_group(8, 8)
        emit_group_fold(8, 8)
        for t_ in range(5, 9):
            conv_tile(*trows[t_])
        tcols_group(2); tcols_group(3)
        emit_group(16, 16)
        emit_group_fold(16, 16)
        for t_ in range(9, 14):
            conv_tile(*trows[t_])
        for g in range(4, 7):
            tcols_group(g)
        emit_group(32, 22)
        emit_group_fold(32, 22)
        for g in range(3, 14):
            emit_trans(g)
        emit_band(1); emit_band(2)

        # ---------------- gather / combine / matmul / epilogue -------------
        for si, (b0, nb) in enumerate(_slices()):
            Q = nb * 9
            npx = nb * 128
            q0 = b0 * 9
            gq = wp.tile([128, SLICE_BLOCKS * 9 * QW], dt.bfloat16, tag="gq",
                         bufs=4)
            gq3 = AP(gq.tensor, gq[:].offset,
                     [gq[:].ap[0], [QW, Q], [1, QW]])
            grp = next(v for v in idxw_tiles.values()
                       if v[0] <= b0 and b0 + nb <= v[0] + v[1])
            lq = (b0 - grp[0]) * 9
            gi = nc.gpsimd.dma_gather(
                out_ap=gq3,
                in_ap=x_quad[:],
                idxs_ap=grp[2][:, lq * 8:(lq + Q) * 8],
                num_idxs=Q * 128,
                num_idxs_reg=Q * 128,
                elem_size=QW,
                single_packet=False,
                queue_num=si % 3,
            )
            for dep in _slice_table_deps(b0, nb):
                add_dep_helper(gi.ins, dep.ins, reason="gather after table")
            if si == 0:
                for dep in group_repls + group_ilvs[-1:]:
                    add_dep_helper(gi.ins, dep.ins,
                                   reason="gathers after all group math")

            # weighted 4-corner combine (in-place over the gathered tile;
            # only the 48 valid channels of each 64-wide slot are touched)
            prod = gq
            gview = AP(gq.tensor, gq[:].offset,
                       [gq[:].ap[0], [QW, Q], [64, 4], [1, C]])
            wb = AP(w4.tensor, w4[:].offset + q0 * 4,
                    [w4[:].ap[0], [4, Q], [1, 4], [0, C]])
            nc.vector.tensor_tensor(gview, gview, wb, op.mult)

            def pview(off):
                return AP(prod.tensor, prod[:].offset + off,
                          [prod[:].ap[0], [QW, Q], [1, C]])
            sa = wp.tile([128, SLICE_BLOCKS * 9 * C], dt.bfloat16, tag="sa")
            # sampt doubles as scratch for the second corner-pair sum; its
            # cols are rewritten only after the final add consumed them
            sampt = wp.tile([128, 4 * SLICE_BLOCKS * 128], dt.bfloat16, tag="sampt")
            sb2 = sampt
            nc.vector.tensor_tensor(sa[:, :Q * C], pview(0), pview(64), op.add)
            nc.vector.tensor_tensor(sb2[:, :Q * C], pview(128), pview(192),
                                    op.add)
            samp = sa
            nc.vector.tensor_tensor(samp[:, :Q * C], sa[:, :Q * C],
                                    sb2[:, :Q * C], op.add)
            for ib in range(nb):
                ps = pp_st.tile([128, 512], dt.bfloat16, tag="ps_st")
                base = ib * 9 * C
                for ch in range(3):
                    nc.tensor.transpose(
                        ps[:, ch * 128:(ch + 1) * 128],
                        samp[:, base + ch * 128: base + (ch + 1) * 128],
                        id128[:])
                nc.tensor.transpose(ps[0:C, 384:512],
                                    samp[:, base + 384: base + 432], id128[:])
                dst02 = AP(sampt.tensor, sampt[:].offset + ib * 128,
                           [sampt[:].ap[0], [npx, 3], [1, 128]])
                nc.scalar.activation(dst02, ps[:, 0:384], act.Copy)
                nc.scalar.activation(sampt[0:C, 3 * npx + ib * 128:
                                           3 * npx + (ib + 1) * 128],
                                     ps[0:C, 384:512], act.Copy)

            # matmul: out[o, px] += dwT_chunk.T @ sampt_chunk
            pso = pp_out.tile([C, SLICE_BLOCKS * 128], dt.float32, tag="ps_out")
            for ch in range(3):
                nc.tensor.matmul(pso[:, :npx], dwt_ap(ch, 128),
                                 sampt[:, ch * npx:(ch + 1) * npx],
                                 start=(ch == 0), stop=False)
            nc.tensor.matmul(pso[:, :npx], dwt_ap(3, C),
                             sampt[0:C, 3 * npx:3 * npx + npx],
                             start=False, stop=True)

            # epilogue: BN(running stats) + residual (DVE) + relu (ACT)
            bno = wp.tile([C, SLICE_BLOCKS * 128], dt.float32, tag="bno")
            nc.scalar.activation(bno[:, :npx], pso[:, :npx], act.Identity,
                                 bias=shift_t[:], scale=scale_t[:])
            nc.vector.tensor_tensor(bno[:, :npx], bno[:, :npx],
                                    x_sb[:, b0 * 128:b0 * 128 + npx], op.add)
            bno2 = wp.tile([C, SLICE_BLOCKS * 128], dt.float32, tag="bno2")
            nc.scalar.activation(bno2[:, :npx], bno[:, :npx], act.Relu)
            nc.sync.dma_start(out_ext[:, b0 * 128:b0 * 128 + npx],
                              bno2[:, :npx])

    nc.compile()
    return nc


def _host_pack(inputs):
    """Weight/constant layout prep (no input-data compute)."""
    bf16 = ml_dtypes.bfloat16
    wa = np.concatenate([inputs['tm_w'], inputs['tr_w'], inputs['mk_w']],
                        axis=0)                       # [15, C, 3, 3]
    bconv = np.concatenate([inputs['tm_b'], inputs['tr_b'],
                            inputs['mk_b']]).astype(np.float32)
    dwr = inputs['dw'].reshape(C, C, 9)               # [o, c, k]
    dwT = np.zeros((512, C), dtype=np.float32)
    dwT[:432] = dwr.transpose(2, 1, 0).reshape(9 * C, C)
    dwt = np.ascontiguousarray(
        dwT.reshape(4, 128, C).transpose(1, 0, 2).reshape(128, 4 * C)
    ).astype(bf16)

    blob16 = np.zeros((128, 282), dtype=bf16)
    for r in range(3):
        blob16[:C, r * 15:(r + 1) * 15] = wa[:, :, r, 0].T
        blob16[C:2 * C, r * 15:(r + 1) * 15] = wa[:, :, r, 1].T
        blob16[:C, 45 + r * 15:45 + (r + 1) * 15] = wa[:, :, r, 2].T
    blob16[:, 90:282] = dwt

    pix = (np.arange(NB)[None, :] * 128 + np.arange(128)[:, None])
    blob32 = np.zeros((128, 131), dtype=np.float32)
    blob32[:, 0:9] = np.tile(_REG[0], (128, 1))
    blob32[:, 9:18] = np.tile(_REG[1], (128, 1))
    blob32[:, 18:72] = (pix // W).astype(np.float32)
    blob32[:, 72:126] = (pix % W).astype(np.float32)
    blob32[:15, 126] = bconv
    blob32[:C, 127] = inputs['gamma'].astype(np.float32)
    blob32[:C, 128] = inputs['beta'].astype(np.float32)
    blob32[:C, 129] = inputs['rmean'].astype(np.float32)
    blob32[:C, 130] = inputs['rvar'].astype(np.float32)
    return dict(blob16=blob16, blob32=blob32)


def kernel(**inputs):
    inputs = {k: np.asarray(v) for k, v in inputs.items()}
    if 'nc' not in _built:
        _built['nc'] = build_nc()
    nc = _built['nc']

    from concourse.bass_utils import run_bass_kernel_spmd
    shared = _host_pack(inputs)
    x = inputs['x'].astype(ml_dtypes.bfloat16)
    in_maps = []
    for i in range(N):
        m = dict(shared)
        m['x'] = np.ascontiguousarray(x[i].reshape(C, HW))
        in_maps.append(m)
    res = run_bass_kernel_spmd(nc, in_maps, core_ids=list(range(N)))
    out = np.stack([res.results[i]['out'].reshape(C, H, W)
                    for i in range(N)])
    return out.astype(np.float32)


# revision 35
# speedup vs baseline: 1.0464x; 1.0464x over previous
"""Trainium2 Bass kernel for nn_AdaptBlockV2 (deformable-conv-v2 block).

Data-parallel over the batch axis: 8 samples -> 8 NeuronCores, one sample
per core. Inside each core:
  A) load x; build zero-padded CHW copy (bf16) for the convs; transpose x to
     HWC (bf16, three band tiles) and write a ROW-MAJOR "quad" gather table
     straight to DRAM with 4 strided DMAs per band (row r = channels of flat
     pixels [r, r+1, r+W, r+W+1] at slots 0..3) -- one indirect-DMA
     descriptor then fetches all 4 bilinear corners of one (pixel, tap).
     Table edge rows are pre-zeroed so zero-weight fetches read 0, not NaN.
  B) 15-channel 3x3 conv (offset transform T, translation tr, modulation
     mask) as 9 PSUM-accumulated matmuls; transpose conv output to
     pixel-major; bulk DVE math for sampling positions py/px, floor via
     floored-mod, corner weights (bilinear x mask x validity), and the flat
     gather index (= table row, no remap needed with the row-major table).
  C) per-slice pipeline: indirect DMA gather -> DVE weighted 4-corner
     combine -> PE transpose of samp to (tap,channel)-major -> matmul with
     dw -> BN (running stats) + residual (DVE add) + ReLU (Scalar engine)
     -> DMA out.

The SWDGE gather descriptor generation on the Pool engine (~8ns/descriptor
x 62208 descriptors ~= 490us) is the hard floor; the prologue is kept to
~25us by minimizing serial Sync-engine DMA issues (the quad table is 10
DMAs instead of ~72; the idx 16-wrap fold is 3 DMAs per group instead of 9
-- queue 0 only needs the indices replicated to partitions 0..31).

kernel(**inputs) takes FULL unsharded inputs, returns the FULL output.
"""
import numpy as np
import ml_dtypes

N, C, H, W = 8, 48, 96, 72
HW = H * W                       # 6912
LEAD = W + 2                     # 74: lead pad rows in the quad table
RQ = 7040                        # quad-table rows (>= HW + W + 2)
QW = 256                         # quad-table row width (512B, dma_gather)
NB = HW // 128                   # 54 pixel blocks
QTOT = NB * 9                    # 486 (block, tap) chunks
PADW = W + 2                     # 74 padded conv row stride
PADLEN = (H + 2) * PADW         # 7252
BN_EPS = 1e-5
CONV_ROWS = 7                    # conv N-tile = 7 image rows = 504 pixels
SLICE_BLOCKS = 4                 # gather/combine slice = 4 pixel blocks
MAXD = 10                        # sample-displacement bound for band deps
BANDS = [(0, 12), (12, 32), (32, 54)]   # x_hwc chunk ranges per table band
DELTA = [0, 1, W, W + 1]                # quad slot pixel offsets

_REG = np.array([[-1, -1, -1, 0, 0, 0, 1, 1, 1],
                 [-1, 0, 1, -1, 0, 1, -1, 0, 1]], dtype=np.float32)

_built = {}


def _slices():
    # big slices first; small 2-block slices at the end so the final
    # gathers (which finish near-simultaneously across the 4 queues)
    # leave only small combine/epilogue tails
    plan = [4] * 11 + [2] * 5
    out = []
    b = 0
    for nb in plan:
        out.append((b, nb))
        b += nb
    assert b == NB
    return out


def build_nc():
    import concourse.bass as bass
    import concourse.bacc as bacc
    import concourse.tile as tile
    from concourse import mybir
    from concourse.bass import AP
    from concourse.masks import make_identity
    from concourse.tile import add_dep_helper
    from contextlib import ExitStack

    dt = mybir.dt
    op = mybir.AluOpType
    act = mybir.ActivationFunctionType

    nc = bacc.Bacc("TRN2", target_bir_lowering=False, debug=False,
                   num_devices=N, dynamic_dma_scratch_size=16384,
                   num_swdge_queues=4)
    x_ext = nc.declare_dram_parameter("x", [C, HW], dt.bfloat16, isOutput=False)
    blob16_ext = nc.declare_dram_parameter("blob16", [128, 282], dt.bfloat16,
                                           isOutput=False)
    blob32_ext = nc.declare_dram_parameter("blob32", [128, 131], dt.float32,
                                           isOutput=False)
    out_ext = nc.declare_dram_parameter("out", [C, HW], dt.float32, isOutput=True)

    x_quad = nc.dram_tensor("x_quad", [RQ, QW], dt.bfloat16)
    idx_dram = nc.dram_tensor("idx_dram", [128 * QTOT], dt.int16)

    with tile.TileContext(nc) as tc, ExitStack() as ctx:
        cp = ctx.enter_context(tc.tile_pool(name="const", bufs=1))
        tp = ctx.enter_context(tc.tile_pool(name="tmp", bufs=1))
        wp = ctx.enter_context(tc.tile_pool(name="work", bufs=2))
        pp_a = ctx.enter_context(tc.tile_pool(name="ps_a", bufs=2, space="PSUM"))
        pp_st = ctx.enter_context(tc.tile_pool(name="ps_st", bufs=2, space="PSUM"))
        pp_out = ctx.enter_context(tc.tile_pool(name="ps_out", bufs=1, space="PSUM"))

        # ---------------- constants / weights to SBUF ----------------
        x_sb = cp.tile([C, HW], dt.bfloat16, tag="x_sb")
        nc.sync.dma_start(x_sb[:, :HW // 2], x_ext[:, :HW // 2])
        nc.sync.dma_start(x_sb[:, HW // 2:], x_ext[:, HW // 2:])
        blob16 = cp.tile([128, 282], dt.bfloat16, tag="blob16")
        nc.sync.dma_start(blob16[:], blob16_ext[:])
        blob32 = cp.tile([128, 131], dt.float32, tag="blob32")
        nc.sync.dma_start(blob32[:], blob32_ext[:])

        b16p = blob16[:].ap[0][0]
        b16o = blob16[:].offset
        b32p = blob32[:].ap[0][0]
        b32o = blob32[:].offset

        def wconvA_ap(r):        # [96, 15] stationary: taps (r,0) + (r,1)
            return AP(blob16.tensor, b16o + r * 15, [[b16p, 2 * C], [1, 15]])

        def wconvB_ap(r):        # [48, 15] stationary: tap (r,2)
            return AP(blob16.tensor, b16o + 45 + r * 15, [[b16p, C], [1, 15]])

        def dwt_ap(ch, nparts):  # [nparts, 48] stationary for dw chunk ch
            return AP(blob16.tensor, b16o + 90 + ch * C, [[b16p, nparts], [1, C]])

        def b32col(col, nparts):
            return AP(blob32.tensor, b32o + col, [[b32p, nparts], [1, 1]])

        bconv_ap = b32col(126, 15)

        id128 = cp.tile([128, 128], dt.bfloat16, tag="id128")
        make_identity(nc, id128[:])
        id48 = id128[0:C, 0:C]
        id16 = id128[0:15, 0:15]

        # Dummy 128-descriptor gather issued first: forces the Pool engine's
        # gather-ucode LOAD_LIB (which barriers on all previously emitted
        # work) to happen immediately against an empty pipeline instead of
        # right before the first real gather.
        dz_idx = tp.tile([128, 8], dt.int16, tag="dz_idx")
        nc.gpsimd.memset(dz_idx[:], 0)
        dz_out = tp.tile([128, 256], dt.bfloat16, tag="dz_out")
        nc.gpsimd.dma_gather(
            out_ap=AP(dz_out.tensor, dz_out[:].offset,
                      [dz_out[:].ap[0], [QW, 1], [1, QW]]),
            in_ap=x_quad[:], idxs_ap=dz_idx[:],
            num_idxs=128, num_idxs_reg=128,
            elem_size=QW, single_packet=False)

        # ---------------- zero the quad-table edge rows ----------------
        # Rows [0, LEAD) and [RQ-127, RQ) can be fetched (clamped idx /
        # out-of-image corners) with zero weight; they must hold 0, not junk.
        ztile = tp.tile([128, 254], dt.bfloat16, tag="ztile")
        nc.vector.memset(ztile[:], 0.0)
        zlow = nc.sync.dma_start(
            out=AP(x_quad, 0, [[148, 128], [1, 148]]),
            in_=ztile[:, :148])
        zhigh = nc.sync.dma_start(
            out=AP(x_quad, (RQ - 127) * QW, [[254, 128], [1, 254]]),
            in_=ztile[:, :254])

        # ---------------- padded CHW copy (bf16) for convs ----------------
        # x_pad2 partitions 0..47 hold padded x; partitions 48..95 hold the
        # same shifted one column left (tap c=1), so one matmul covers two
        # taps with a [96, 15] stationary. Only pad cells are memset.
        x_pad2 = cp.tile([2 * C, PADLEN], dt.bfloat16, tag="x_pad2")
        xp_p = x_pad2[:].ap[0][0]
        xp_o = x_pad2[:].offset
        nc.vector.memset(x_pad2[0:C, 0:PADW + 1], 0.0)
        nc.vector.memset(
            AP(x_pad2.tensor, xp_o + 2 * PADW - 1,
               [[xp_p, C], [PADW, H - 1], [1, 2]]), 0.0)
        nc.vector.memset(x_pad2[0:C, (H + 1) * PADW - 1:PADLEN], 0.0)
        nc.vector.tensor_copy(
            AP(x_pad2.tensor, xp_o + PADW + 1, [[xp_p, C], [PADW, H // 2], [1, W]]),
            x_sb[:, :HW // 2])
        nc.vector.tensor_copy(
            AP(x_pad2.tensor, xp_o + (H // 2 + 1) * PADW + 1,
               [[xp_p, C], [PADW, H // 2], [1, W]]),
            x_sb[:, HW // 2:])
        nc.sync.dma_start(
            out=AP(x_pad2.tensor, xp_o + C * xp_p,
                   [[xp_p, C], [1, PADLEN - 1]]),
            in_=AP(x_pad2.tensor, xp_o + 1, [[xp_p, C], [1, PADLEN - 1]]))

        # ---------------- x -> HWC (bf16) via PE transposes ----------------
        # One tile per table band so band writes only dep on their own chunks.
        hwc_tiles = [cp.tile([128, (k1 - k0) * C], dt.bfloat16,
                             name=f"x_hwc{i}", tag=f"x_hwc{i}")
                     for i, (k0, k1) in enumerate(BANDS)]

        def hwc_of(chunk):
            for (k0, k1), t in zip(BANDS, hwc_tiles):
                if k0 <= chunk < k1:
                    return t, k0
            raise AssertionError

        def emit_trans(g):                    # 4 blocks per PSUM tile
            nblk = min(4, NB - g * 4)
            ps = pp_a.tile([128, 4 * C], dt.bfloat16, name="psA", tag="psA")
            for j in range(nblk):
                b = g * 4 + j
                nc.tensor.transpose(ps[:, j * C:(j + 1) * C],
                                    x_sb[:, b * 128:(b + 1) * 128], id48[:])
            t, k0 = hwc_of(g * 4)
            nc.scalar.activation(
                t[:, (g * 4 - k0) * C:(g * 4 - k0 + nblk) * C],
                ps[:, :nblk * C], act.Copy)

        # ---------------- quad table: 4 slot DMAs per band -----------------
        band_ins = {}

        def emit_band(bi):
            k0, k1 = BANDS[bi]
            nk = k1 - k0
            t, tk0 = hwc_of(k0)
            ws = []
            for j, dj in enumerate(DELTA):
                base = (LEAD - dj) * QW + 64 * j
                w = nc.sync.dma_start(
                    out=AP(x_quad, base + k0 * 128 * QW,
                           [[QW, 128], [128 * QW, nk], [1, C]]),
                    in_=AP(t.tensor, t[:].offset + (k0 - tk0) * C,
                           [t[:].ap[0], [C, nk], [1, C]]))
                if bi == 0:
                    add_dep_helper(w.ins, zlow.ins, reason="slot after zero")
                if bi == 2:
                    add_dep_helper(w.ins, zhigh.ins, reason="slot after zero")
                ws.append(w)
            band_ins[bi] = ws

        def _slice_table_deps(b0, nb):
            y_lo = (b0 * 128) // W
            y_hi = ((b0 + nb) * 128 - 1) // W
            r_lo = max(0, LEAD + (y_lo - MAXD) * W)
            r_hi = min(RQ - 1, LEAD + (y_hi + MAXD + 1) * W + W - 1)
            deps = []
            if r_lo < LEAD:
                deps.append(zlow)
            if r_hi >= RQ - 127:
                deps.append(zhigh)
            for bi, (k0, k1) in enumerate(BANDS):
                if r_lo <= k1 * 128 + W + 1 and r_hi >= k0 * 128 + 1:
                    deps.extend(band_ins[bi])
            return deps

        # bn scale' = gamma * rsqrt(rvar+eps); shift' = beta - rmean*scale'
        def emit_bn():
            veps = tp.tile([C, 1], dt.float32, tag="veps")
            nc.vector.tensor_scalar(veps[:], b32col(130, C), BN_EPS, None, op.add)
            vsq = tp.tile([C, 1], dt.float32, tag="vsq")
            nc.scalar.activation(vsq[:], veps[:], act.Sqrt)
            vri = tp.tile([C, 1], dt.float32, tag="vri")
            nc.vector.reciprocal(vri[:], vsq[:])
            scale_t = cp.tile([C, 1], dt.float32, tag="scale")
            nc.vector.tensor_tensor(scale_t[:], b32col(127, C), vri[:], op.mult)
            vms = tp.tile([C, 1], dt.float32, tag="vms")
            nc.vector.tensor_tensor(vms[:], b32col(129, C), scale_t[:], op.mult)
            shift_t = cp.tile([C, 1], dt.float32, tag="shift")
            nc.vector.tensor_tensor(shift_t[:], b32col(128, C), vms[:], op.subtract)
            return scale_t, shift_t

        # ---------------- convs: 15ch 3x3 via 6 accumulated matmuls --------
        conv_sb = cp.tile([15, HW], dt.bfloat16, tag="conv_sb")
        trows = [(t * CONV_ROWS, min(CONV_ROWS, H - t * CONV_ROWS))
                 for t in range((H + CONV_ROWS - 1) // CONV_ROWS)]

        def conv_tile(r0, nr):
            psc = pp_a.tile([15, CONV_ROWS * W], dt.float32, name="psA2",
                            tag="psA2", bufs=3)
            npx = nr * W
            for r in range(3):
                rhsA = AP(x_pad2.tensor, xp_o + (r0 + r) * PADW,
                          [[xp_p, 2 * C], [PADW, nr], [1, W]])
                nc.tensor.matmul(psc[:, :npx], wconvA_ap(r), rhsA,
                                 start=(r == 0), stop=False)
                rhsB = AP(x_pad2.tensor, xp_o + (r0 + r) * PADW + 2,
                          [[xp_p, C], [PADW, nr], [1, W]])
                nc.tensor.matmul(psc[:, :npx], wconvB_ap(r), rhsB,
                                 start=False, stop=(r == 2))
            nc.scalar.activation(conv_sb[:, r0 * W:r0 * W + npx], psc[:, :npx],
                                 act.Identity, bias=bconv_ap)

        # conv output -> pixel-major (tcols), per 8-block group
        tcols = cp.tile([128, NB * 15], dt.float32, tag="tcols")

        def tcols_group(g):
            nblk = min(8, NB - g * 8)
            ps = pp_a.tile([128, 8 * 16], dt.bfloat16, name="psA", tag="psA")
            for j in range(nblk):
                b = g * 8 + j
                nc.tensor.transpose(ps[:, j * 16:j * 16 + 15],
                                    conv_sb[:, b * 128:(b + 1) * 128],
                                    id16[:])
            nc.scalar.activation(tcols[:, g * 8 * 15:(g * 8 + nblk) * 15],
                                 AP(ps.tensor, ps[:].offset,
                                    [ps[:].ap[0], [16, nblk], [1, 15]]),
                                 act.Copy)

        # ---------------- bulk offset / weight / index math ----------------
        def plane(tag):
            return tp.tile([128, QTOT], dt.float32, name=tag, tag=tag)

        py = plane("py"); px = plane("px")
        t1 = plane("t1"); t2 = plane("t2"); t3 = plane("t3")
        fy = plane("fy"); fx = plane("fx")
        y0 = plane("y0"); x0 = plane("x0")
        wy0 = py; wy1 = px                       # reuse dead slots (disjoint cols)
        wx0 = plane("wx0"); wx1 = plane("wx1")
        idxf = fy                                # reuse dead slot
        cvt_i = tp.tile([128, QTOT], dt.int32, name="cvt_i", tag="cvt_i")
        w4 = cp.tile([128, QTOT * 4], dt.bfloat16, tag="w4")
        idx_t = cp.tile([128, QTOT], dt.int16, tag="idx")
        fold = cp.tile([16, 8 * QTOT], dt.int16, tag="fold")
        # one idxw tile per offset-math group, so a slice gather depends only
        # on its own group's fold (Tile tracks whole-tile versions)
        idxw_tiles = {}
        group_repls = []
        group_ilvs = []
        groups = [(0, 4), (4, 4), (8, 8), (16, 16), (32, 22)]

        def emit_group(g0, gn):
            nq = gn * 9
            c0 = g0 * 9
            sl = slice(c0, c0 + nq)

            def tcol_b(ch):
                return AP(tcols.tensor, tcols[:].offset + ch + g0 * 15,
                          [tcols[:].ap[0], [15, gn], [0, 9]])

            def reg_b(roff):
                return AP(blob32.tensor, b32o + roff,
                          [[b32p, 128], [0, gn], [1, 9]])

            def coord_b(coff):
                return AP(blob32.tensor, b32o + coff + g0,
                          [[b32p, 128], [1, gn], [0, 9]])

            tt = nc.vector.tensor_tensor
            ts = nc.vector.tensor_scalar
            tt(t1[:, sl], reg_b(0), tcol_b(0), op.mult)
            tt(t2[:, sl], reg_b(9), tcol_b(1), op.mult)
            tt(t3[:, sl], t1[:, sl], t2[:, sl], op.add)
            tt(t1[:, sl], t3[:, sl], tcol_b(4), op.add)
            tt(py[:, sl], t1[:, sl], coord_b(18), op.add)
            tt(t1[:, sl], reg_b(0), tcol_b(2), op.mult)
            tt(t2[:, sl], reg_b(9), tcol_b(3), op.mult)
            tt(t3[:, sl], t1[:, sl], t2[:, sl], op.add)
            tt(t1[:, sl], t3[:, sl], tcol_b(5), op.add)
            tt(px[:, sl], t1[:, sl], coord_b(72), op.add)

            # floor(v) = int(v) - (v < int(v)); rounding-mode agnostic
            for (v, fl, fr) in ((py, y0, fy), (px, x0, fx)):
                nc.vector.tensor_copy(cvt_i[:, sl], v[:, sl])
                nc.vector.tensor_copy(t1[:, sl], cvt_i[:, sl])
                tt(t2[:, sl], v[:, sl], t1[:, sl], op.is_lt)
                tt(fl[:, sl], t1[:, sl], t2[:, sl], op.subtract)
                tt(fr[:, sl], v[:, sl], fl[:, sl], op.subtract)

            mk_b = AP(tcols.tensor, tcols[:].offset + 6 + g0 * 15,
                      [tcols[:].ap[0], [15, gn], [1, 9]])
            ts(t1[:, sl], y0[:, sl], 0.0, None, op.is_ge)
            ts(t2[:, sl], y0[:, sl], float(H - 1), None, op.is_le)
            tt(t3[:, sl], t1[:, sl], t2[:, sl], op.mult)
            ts(t1[:, sl], fy[:, sl], -1.0, 1.0, op.mult, op.add)
            tt(wy0[:, sl], t1[:, sl], t3[:, sl], op.mult)
            ts(t1[:, sl], y0[:, sl], -1.0, None, op.is_ge)
            ts(t2[:, sl], y0[:, sl], float(H - 2), None, op.is_le)
            tt(t3[:, sl], t1[:, sl], t2[:, sl], op.mult)
            tt(wy1[:, sl], fy[:, sl], t3[:, sl], op.mult)
            ts(t1[:, sl], x0[:, sl], 0.0, None, op.is_ge)
            ts(t2[:, sl], x0[:, sl], float(W - 1), None, op.is_le)
            tt(t3[:, sl], t1[:, sl], t2[:, sl], op.mult)
            ts(t1[:, sl], fx[:, sl], -1.0, 1.0, op.mult, op.add)
            tt(wx0[:, sl], t1[:, sl], t3[:, sl], op.mult)
            ts(t1[:, sl], x0[:, sl], -1.0, None, op.is_ge)
            ts(t2[:, sl], x0[:, sl], float(W - 2), None, op.is_le)
            tt(t3[:, sl], t1[:, sl], t2[:, sl], op.mult)
            tt(wx1[:, sl], fx[:, sl], t3[:, sl], op.mult)
            tt(wy0[:, sl], wy0[:, sl], mk_b, op.mult)
            tt(wy1[:, sl], wy1[:, sl], mk_b, op.mult)

            def w4_slot(j):
                return AP(w4.tensor, w4[:].offset + j + c0 * 4,
                          [w4[:].ap[0], [4, nq]])
            tt(w4_slot(0), wy0[:, sl], wx0[:, sl], op.mult)
            tt(w4_slot(1), wy0[:, sl], wx1[:, sl], op.mult)
            tt(w4_slot(2), wy1[:, sl], wx0[:, sl], op.mult)
            tt(w4_slot(3), wy1[:, sl], wx1[:, sl], op.mult)

            # gather index = table row r = clip(LEAD + y0*W + x0, 0, RQ-1)
            nc.vector.scalar_tensor_tensor(idxf[:, sl], y0[:, sl], float(W),
                                           x0[:, sl], op.mult, op.add)
            ts(idxf[:, sl], idxf[:, sl], float(LEAD), 0.0, op.add, op.max)
            ts(idxf[:, sl], idxf[:, sl], float(RQ - 1), None, op.min)
            nc.vector.tensor_copy(idx_t[:, sl], idxf[:, sl])

            # 16-wrap fold DRAM round-trip (Sync only; DVE interleave comes
            # later, after every group's math, so it never stalls the stream)
            wr = nc.sync.dma_start(
                out=AP(idx_dram, c0, [[QTOT, 128], [1, nq]]),
                in_=idx_t[:, sl])
            rdf = nc.sync.dma_start(
                out=AP(fold.tensor, fold[:].offset + c0,
                       [fold[:].ap[0], [QTOT, 8], [1, nq]]),
                in_=AP(idx_dram, c0, [[QTOT, 16], [16 * QTOT, 8], [1, nq]]))
            add_dep_helper(rdf.ins, wr.ins, sync=False, reason="idx fold rt")

        def emit_group_fold(g0, gn):
            nq = gn * 9
            c0 = g0 * 9
            idxw = cp.tile([128, nq * 8], dt.int16, name=f"idxw{g0}",
                           tag=f"idxw{g0}")
            idxw_tiles[g0] = (g0, gn, idxw)
            i16 = idxw[0:16, :]
            ic = nc.gpsimd.tensor_copy(
                AP(i16.tensor, i16.offset,
                   [i16.ap[0], [1, 8], [8, nq]]),
                AP(fold.tensor, fold[:].offset + c0,
                   [fold[:].ap[0], [QTOT, 8], [1, nq]]))
            group_ilvs.append(ic)
            # queue q reads idx partitions [32q, 32q+32): replicate the
            # 16-row wrap to all of 0..127 with 7 parallel DMAs
            for k in range(1, 8):
                r = nc.sync.dma_start(out=idxw[16 * k:16 * (k + 1), :],
                                      in_=idxw[0:16, :])
            group_repls.append(r)

        # ---------------- emission schedule ----------------
        # All offset-math groups complete BEFORE the first gather launches:
        # DVE ops that execute while a Pool gather is in flight can stall
        # until the gather retires (observed on HW), so the gather window is
        # kept to combine/epilogue ops that are proven immune. PE runs the
        # convs first (they feed tcols -> group math), then the x_hwc
        # transposes that feed the table-band writes.
        for g in range(3):
            emit_trans(g)
        emit_band(0)
        conv_tile(*trows[0]); conv_tile(*trows[1]); conv_tile(*trows[2])
        tcols_group(0)
        emit_group(0, 4)
        emit_group_fold(0, 4)
        emit_group(4, 4)
        emit_group_fold(4, 4)
        scale_t, shift_t = emit_bn()
        conv_tile(*trows[3]); conv_tile(*trows[4])
        tcols_group(1)
        emit_group(8, 8)
        emit_group_fold(8, 8)
        for t_ in range(5, 9):
            conv_tile(*trows[t_])
        tcols_group(2); tcols_group(3)
        emit_group(16, 16)
        emit_group_fold(16, 16)
        for t_ in range(9, 14):
            conv_tile(*trows[t_])
        for g in range(4, 7):
            tcols_group(g)
        emit_group(32, 22)
        emit_group_fold(32, 22)
        for g in range(3, 14):
            emit_trans(g)
        emit_band(1); emit_band(2)

        # ---------------- gather / combine / matmul / epilogue -------------
        for si, (b0, nb) in enumerate(_slices()):
            Q = nb * 9
            npx = nb * 128
            q0 = b0 * 9
            gq = wp.tile([128, SLICE_BLOCKS * 9 * QW], dt.bfloat16, tag="gq",
                         bufs=4)
            gq3 = AP(gq.tensor, gq[:].offset,
                     [gq[:].ap[0], [QW, Q], [1, QW]])
            grp = next(v for v in idxw_tiles.values()
                       if v[0] <= b0 and b0 + nb <= v[0] + v[1])
            lq = (b0 - grp[0]) * 9
            gi = nc.gpsimd.dma_gather(
                out_ap=gq3,
                in_ap=x_quad[:],
                idxs_ap=grp[2][:, lq * 8:(lq + Q) * 8],
                num_idxs=Q * 128,
                num_idxs_reg=Q * 128,
                elem_size=QW,
                single_packet=False,
                queue_num=si % 3,
            )
            for dep in _slice_table_deps(b0, nb):
                add_dep_helper(gi.ins, dep.ins, reason="gather after table")
            if si == 0:
                for dep in group_repls + group_ilvs[-1:]:
                    add_dep_helper(gi.ins, dep.ins,
                                   reason="gathers after all group math")

            # weighted 4-corner combine (in-place over the gathered tile;
            # only the 48 valid channels of each 64-wide slot are touched)
            prod = gq
            gview = AP(gq.tensor, gq[:].offset,
                       [gq[:].ap[0], [QW, Q], [64, 4], [1, C]])
            wb = AP(w4.tensor, w4[:].offset + q0 * 4,
                    [w4[:].ap[0], [4, Q], [1, 4], [0, C]])
            nc.vector.tensor_tensor(gview, gview, wb, op.mult)

            def pview(off):
                return AP(prod.tensor, prod[:].offset + off,
                          [prod[:].ap[0], [QW, Q], [1, C]])
            sa = wp.tile([128, SLICE_BLOCKS * 9 * C], dt.bfloat16, tag="sa")
            # sampt doubles as scratch for the second corner-pair sum; its
            # cols are rewritten only after the final add consumed them
            sampt = wp.tile([128, 4 * SLICE_BLOCKS * 128], dt.bfloat16, tag="sampt")
            sb2 = sampt
            nc.vector.tensor_tensor(sa[:, :Q * C], pview(0), pview(64), op.add)
            nc.vector.tensor_tensor(sb2[:, :Q * C], pview(128), pview(192),
                                    op.add)
            samp = sa
            nc.vector.tensor_tensor(samp[:, :Q * C], sa[:, :Q * C],
                                    sb2[:, :Q * C], op.add)
            for ib in range(nb):
                ps = pp_st.tile([128, 512], dt.bfloat16, tag="ps_st")
                base = ib * 9 * C
                for ch in range(3):
                    nc.tensor.transpose(
                        ps[:, ch * 128:(ch + 1) * 128],
                        samp[:, base + ch * 128: base + (ch + 1) * 128],
                        id128[:])
                nc.tensor.transpose(ps[0:C, 384:512],
                                    samp[:, base + 384: base + 432], id128[:])
                dst02 = AP(sampt.tensor, sampt[:].offset + ib * 128,
                           [sampt[:].ap[0], [npx, 3], [1, 128]])
                nc.scalar.activation(dst02, ps[:, 0:384], act.Copy)
                nc.scalar.activation(sampt[0:C, 3 * npx + ib * 128:
                                           3 * npx + (ib + 1) * 128],
                                     ps[0:C, 384:512], act.Copy)

            # matmul: out[o, px] += dwT_chunk.T @ sampt_chunk
            pso = pp_out.tile([C, SLICE_BLOCKS * 128], dt.float32, tag="ps_out")
            for ch in range(3):
                nc.tensor.matmul(pso[:, :npx], dwt_ap(ch, 128),
                                 sampt[:, ch * npx:(ch + 1) * npx],
                                 start=(ch == 0), stop=False)
            nc.tensor.matmul(pso[:, :npx], dwt_ap(3, C),
                             sampt[0:C, 3 * npx:3 * npx + npx],
                             start=False, stop=True)

            # epilogue: BN(running stats) + residual (DVE) + relu (ACT)
            bno = wp.tile([C, SLICE_BLOCKS * 128], dt.float32, tag="bno")
            nc.scalar.activation(bno[:, :npx], pso[:, :npx], act.Identity,
                                 bias=shift_t[:], scale=scale_t[:])
            nc.vector.tensor_tensor(bno[:, :npx], bno[:, :npx],
                                    x_sb[:, b0 * 128:b0 * 128 + npx], op.add)
            bno2 = wp.tile([C, SLICE_BLOCKS * 128], dt.float32, tag="bno2")
            nc.scalar.activation(bno2[:, :npx], bno[:, :npx], act.Relu)
            nc.sync.dma_start(out_ext[:, b0 * 128:b0 * 128 + npx],
                              bno2[:, :npx])

    nc.compile()
    return nc


def _host_pack(inputs):
    """Weight/constant layout prep (no input-data compute)."""
    bf16 = ml_dtypes.bfloat16
    wa = np.concatenate([inputs['tm_w'], inputs['tr_w'], inputs['mk_w']],
                        axis=0)                       # [15, C, 3, 3]
    bconv = np.concatenate([inputs['tm_b'], inputs['tr_b'],
                            inputs['mk_b']]).astype(np.float32)
    dwr = inputs['dw'].reshape(C, C, 9)               # [o, c, k]
    dwT = np.zeros((512, C), dtype=np.float32)
    dwT[:432] = dwr.transpose(2, 1, 0).reshape(9 * C, C)
    dwt = np.ascontiguousarray(
        dwT.reshape(4, 128, C).transpose(1, 0, 2).reshape(128, 4 * C)
    ).astype(bf16)

    blob16 = np.zeros((128, 282), dtype=bf16)
    for r in range(3):
        blob16[:C, r * 15:(r + 1) * 15] = wa[:, :, r, 0].T
        blob16[C:2 * C, r * 15:(r + 1) * 15] = wa[:, :, r, 1].T
        blob16[:C, 45 + r * 15:45 + (r + 1) * 15] = wa[:, :, r, 2].T
    blob16[:, 90:282] = dwt

    pix = (np.arange(NB)[None, :] * 128 + np.arange(128)[:, None])
    blob32 = np.zeros((128, 131), dtype=np.float32)
    blob32[:, 0:9] = np.tile(_REG[0], (128, 1))
    blob32[:, 9:18] = np.tile(_REG[1], (128, 1))
    blob32[:, 18:72] = (pix // W).astype(np.float32)
    blob32[:, 72:126] = (pix % W).astype(np.float32)
    blob32[:15, 126] = bconv
    blob32[:C, 127] = inputs['gamma'].astype(np.float32)
    blob32[:C, 128] = inputs['beta'].astype(np.float32)
    blob32[:C, 129] = inputs['rmean'].astype(np.float32)
    blob32[:C, 130] = inputs['rvar'].astype(np.float32)
    return dict(blob16=blob16, blob32=blob32)


def kernel(**inputs):
    inputs = {k: np.asarray(v) for k, v in inputs.items()}
    if 'nc' not in _built:
        _built['nc'] = build_nc()
    nc = _built['nc']

    from concourse.bass_utils import run_bass_kernel_spmd
    shared = _host_pack(inputs)
    x = inputs['x'].astype(ml_dtypes.bfloat16)
    in_maps = []
    for i in range(N):
        m = dict(shared)
        m['x'] = np.ascontiguousarray(x[i].reshape(C, HW))
        in_maps.append(m)
    res = run_bass_kernel_spmd(nc, in_maps, core_ids=list(range(N)))
    out = np.stack([res.results[i]['out'].reshape(C, H, W)
                    for i in range(N)])
    return out.astype(np.float32)
